# revision 55
# baseline (speedup 1.0000x reference)
"""Distributed multi-head attention for 8 TRN2 NeuronCores — v5.1 (AllToAll).

Problem: x[2,2048,1024] -> QKV proj (w_qkv[3072,1024]) -> 16-head SDPA ->
out proj (w_proj[1024,1024] + b_proj) -> [2,2048,1024].

Sharding: 2 heads per core (head-parallel; both batches on every core).

The Scalar engine's EXP stream is the hard floor (16.8M softmax elements
per core at 1 elem/cycle/lane ~= 120us); everything else hides under it:
  - Phase A pre-computes only K(b0) + Q(u0) + V(c0); the rest of b0's QKV
    and ALL of b1's QKV drain as PE filler into the EXP-bound attention
    units (attention starts ~20us in).
  - S/O matmuls are software-pipelined ACROSS unit boundaries (S of the
    next group is emitted before O of the previous) so the EXP stream
    never bubbles at unit edges.
  - Per-PAIR AllToAll: each core sends each peer only the 64-token slices
    the peer needs (8x less traffic than AllGather, 4 collectives).
  - Normalization per head, pipelined: den copy -> reciprocal_approx_fast
    -> gpsimd partition_broadcast -> multiply -> half DMA, with head 0's
    chain hidden under head 1's last O matmuls. No PE involvement.
  - Out-proj c_mm(p) drains 2+ units after A2A(p) is issued (collective
    latency is 10-25us: barrier + peer skew); pairs 2,3 run in the tail.
"""
import sys, os, types
import numpy as np

if "/opt/trn_rl_repo" not in sys.path and os.path.isdir("/opt/trn_rl_repo"):
    sys.path.append("/opt/trn_rl_repo")

import concourse.bass as bass
import concourse.mybir as mybir
import concourse.tile as tile
from concourse import bacc
from concourse.bass_utils import run_bass_kernel_spmd

F32 = mybir.dt.float32
F16 = mybir.dt.float16
BF16 = mybir.dt.bfloat16
EXP = mybir.ActivationFunctionType.Exp

NCORES = 8
B, N, C, H, D = 2, 2048, 1024, 16, 64
NT = B * N
KT = C // 128
QC = 512
NU = NT // QC          # 8 units of 512 query tokens
NMT = N // 128         # 16 key tiles per batch
SCALE = 1.0 / 8.0
GRP = 2
XCH = 1024
NPAIR = NU // 2        # 4 unit pairs -> 4 AllToAlls

TRACE = False
LAST_EXEC_NS = None

_NC = None


def _install_ntff_hook():
    if "antenv.axon_hooks" in sys.modules:
        return
    try:
        import antenv
        from trn_agent_boot.trn_boot import _ntff_profile_via_ctypes
        mod = types.ModuleType("antenv.axon_hooks")
        _hook = [None]
        mod.set_axon_ntff_profile_hook = lambda h: _hook.__setitem__(0, h)
        mod.get_axon_ntff_profile_hook = lambda: _hook[0]
        sys.modules["antenv.axon_hooks"] = mod
        antenv.axon_hooks = mod
        mod.set_axon_ntff_profile_hook(
            _ntff_profile_via_ctypes("/opt/axon/libaxon_pjrt.so"))
    except Exception:
        pass


def _build():
    nc = bacc.Bacc("TRN2", target_bir_lowering=False, debug=False,
                   num_devices=NCORES)
    xT_ext = nc.dram_tensor("xT", [C, NT], BF16, kind="ExternalInput").ap()
    wT_ext = nc.dram_tensor("wT", [C, 384], BF16, kind="ExternalInput").ap()
    wpT_ext = nc.dram_tensor("wpT", [C, C], BF16, kind="ExternalInput").ap()
    bias_ext = nc.dram_tensor("bias", [1, C], F32, kind="ExternalInput").ap()
    idn_ext = nc.dram_tensor("idn", [128, 128], BF16, kind="ExternalInput").ap()
    out_ext = nc.dram_tensor("out", [NT // NCORES, C], F16,
                             kind="ExternalOutput").ap()
    a2a_warm_in = nc.dram_tensor("a2a_warm_in", [NCORES, 16, 8], BF16).ap()
    a2a_warm_out = nc.dram_tensor("a2a_warm_out", [NCORES, 16, 8], BF16).ap()
    a2a_in = [nc.dram_tensor(f"a2a_in{p}", [NCORES, 128, 128], BF16).ap()
              for p in range(NPAIR)]
    a2a_out = [nc.dram_tensor(f"a2a_out{p}", [NCORES, 128, 128], BF16).ap()
               for p in range(NPAIR)]

    xT_v = xT_ext.rearrange("(kt p) n -> p kt n", p=128)
    wT_v = wT_ext.rearrange("(kt p) f -> p kt f", p=128)
    wpT_v = wpT_ext.rearrange("(kt p) f -> p kt f", p=128)
    GROUPS = [list(range(NCORES))]

    with tile.TileContext(nc) as tc:
        with (
            tc.tile_pool(name="const", bufs=1) as cpool,
            tc.tile_pool(name="resid", bufs=1) as rpool,
            tc.tile_pool(name="vtmp", bufs=2) as vpool,
            tc.tile_pool(name="pexp", bufs=4) as ppool,
            tc.tile_pool(name="denp", bufs=2) as denpool,
            tc.tile_pool(name="onrm", bufs=3) as onpool,
            tc.tile_pool(name="lhsp", bufs=2) as lhspool,
            tc.tile_pool(name="postp", bufs=2) as postpool,
        ):
            # ---- constants / weights -------------------------------------
            wT_sb = cpool.tile([128, KT, 384], BF16)
            # K columns first so attention's S matmuls unblock earliest
            nc.sync.dma_start(wT_sb[:, :, 128:256], wT_v[:, :, 128:256])
            nc.sync.dma_start(wT_sb[:, :, 0:128], wT_v[:, :, 0:128])
            nc.sync.dma_start(wT_sb[:, :, 256:384], wT_v[:, :, 256:384])
            idn = cpool.tile([128, 128], BF16)
            nc.sync.dma_start(idn[:], idn_ext[:])
            wp_sb = cpool.tile([128, KT, C], BF16)
            bias_sb = cpool.tile([1, C], F32)
            nc.sync.dma_start(bias_sb[:], bias_ext[:])
            bias_bc2 = cpool.tile([128, C], F32)
            nc.gpsimd.partition_broadcast(bias_bc2[:], bias_sb[:])
            # preload ACT exp tables while DMAs run
            dum_in = cpool.tile([1, 8], F32)
            nc.gpsimd.memset(dum_in[:], 0.0)
            dum_out = cpool.tile([1, 8], BF16)
            nc.scalar.activation(dum_out[:], dum_in[:], EXP, scale=1.0)
            # warm-up collective: absorbs ncfw init + first-op overhead
            # ahead of the first real AllToAll
            warm2 = cpool.tile([16, NCORES, 8], BF16)
            nc.gpsimd.memset(warm2[:], 0.0)
            nc.sync.dma_start(
                a2a_warm_in.rearrange("d p q -> p d q"), warm2[:])
            nc.gpsimd.collective_compute(
                "AllToAll", mybir.AluOpType.bypass,
                replica_groups=GROUPS,
                ins=[a2a_warm_in[:]], outs=[a2a_warm_out[:]])

            # ---- residents ----------------------------------------------
            qT_sb = rpool.tile([128, NT], BF16)
            kT_sb = rpool.tile([128, NT], BF16)
            v_sb = rpool.tile([128, NT // 128, 130], BF16)
            nc.gpsimd.memset(v_sb[:, :, 64], 1.0)
            nc.gpsimd.memset(v_sb[:, :, 129], 1.0)
            stage = rpool.tile([64, 2 * NU, QC], BF16)

            # ---- input loads --------------------------------------------
            x_tiles = []
            for nch in range(NT // XCH):
                x_t = rpool.tile([128, KT, XCH], BF16, name=f"x_{nch}")
                x_tiles.append(x_t)
            for kt in range(KT):
                nc.sync.dma_start(x_tiles[0][:, kt, 0:QC],
                                  xT_v[:, kt, 0:QC])
            nc.sync.dma_start(x_tiles[0][:, :, QC:XCH],
                              xT_v[:, :, QC:XCH])
            nc.sync.dma_start(x_tiles[1][:, :, 0:QC],
                              xT_v[:, :, XCH:XCH + QC])
            nc.sync.dma_start(x_tiles[1][:, :, QC:XCH],
                              xT_v[:, :, XCH + QC:2 * XCH])
            # b1's chunks load from early-attention drains instead (keeps
            # startup HBM bandwidth for c0/c1, which gate unit 0)

            # ---- QKV group emitter (split into 2 sub-emissions) ---------
            def qkv_subs(qpool, trpool, bat, tok0, ft):
                """4 quarter-emissions (2 kt each) so drains interleave
                into the EXP-bound attention stream without punching
                holes in it."""
                ncol = bat * N + tok0
                nch = ncol // XCH
                off = ncol % XCH
                x_t = x_tiles[nch]
                box = {}

                def mk(k0):
                    def emit():
                        xs = x_t[:, :, off:off + QC]
                        if k0 == 0:
                            box["ps"] = qpool.tile(
                                [128, QC], F32, tag="qkv",
                                name=f"qkv_{ncol}_{ft}")
                        ps = box["ps"]
                        for kt in range(k0, k0 + 2):
                            nc.tensor.matmul(
                                ps[:],
                                wT_sb[:, kt, ft * 128:(ft + 1) * 128],
                                xs[:, kt, :], start=(kt == 0),
                                stop=False)
                    return emit

                def sub3():
                    xs = x_t[:, :, off:off + QC]
                    ps = box["ps"]
                    for kt in range(6, KT):
                        nc.tensor.matmul(
                            ps[:], wT_sb[:, kt, ft * 128:(ft + 1) * 128],
                            xs[:, kt, :], start=False, stop=(kt == KT - 1))
                    if ft == 0:
                        nc.vector.tensor_copy(
                            out=qT_sb[:, ncol:ncol + QC], in_=ps[:])
                    elif ft == 1:
                        nc.vector.tensor_copy(
                            out=kT_sb[:, ncol:ncol + QC], in_=ps[:])
                    else:
                        vt = vpool.tile([128, QC], BF16, tag="vt",
                                        name=f"vt_{ncol}")
                        nc.vector.tensor_copy(out=vt[:], in_=ps[:])
                        mtg0 = ncol // 128
                        trp = trpool.tile([128, 4, 128], BF16, tag="tr",
                                          name=f"tr_{mtg0}")
                        for t in range(4):
                            nc.tensor.transpose(
                                trp[:, t, :],
                                vt[:, t * 128:(t + 1) * 128], idn[:])
                        nc.vector.tensor_copy(
                            out=v_sb[:, mtg0:mtg0 + 4, 0:64],
                            in_=trp[:, :, 0:64])
                        nc.vector.tensor_copy(
                            out=v_sb[:, mtg0:mtg0 + 4, 65:129],
                            in_=trp[:, :, 64:128])

                def sub0():
                    mk(0)()
                    mk(2)()

                def sub1():
                    mk(4)()
                    sub3()

                return [sub0, sub1]

            # ---- out-projection (destination side) ----------------------
            pair_tiles = {}

            def c_load(p):
                lhs = lhspool.tile([128, NCORES, 128], BF16, tag="lhs",
                                   name=f"lhs_{p}")
                pair_tiles[p] = lhs
                nc.sync.dma_start(lhs[:],
                                  a2a_out[p].rearrange("s p q -> p s q"))

            def c_mm_subs(p, cpsum):
                box = {}

                def mk(half, lo, hi, fin):
                    def emit():
                        lhs = pair_tiles[p]
                        if "ob" not in box:
                            box["ob"] = postpool.tile(
                                [128, C], F16, tag="ob", name=f"ob_{p}")
                        if (half, "pc") not in box:
                            box[(half, "pc")] = cpsum.tile(
                                [128, QC], F32, tag="c",
                                name=f"c_{p}_{half}")
                        pc = box[(half, "pc")]
                        for s in range(lo, hi):
                            nc.tensor.matmul(
                                pc[:], lhs[:, s, :],
                                wp_sb[:, s, half * QC:(half + 1) * QC],
                                start=(s == 0), stop=(s == NCORES - 1))
                        if fin:
                            nc.vector.tensor_tensor(
                                box["ob"][:, half * QC:(half + 1) * QC],
                                pc[:],
                                bias_bc2[:, half * QC:(half + 1) * QC],
                                mybir.AluOpType.add)
                            # per-half output DMA: half 0's store overlaps
                            # half 1's matmuls instead of serializing after
                            nc.sync.dma_start(
                                out_ext[p * 128:(p + 1) * 128,
                                        half * QC:(half + 1) * QC],
                                box["ob"][:, half * QC:(half + 1) * QC])
                    return emit

                return [mk(0, 0, 4, False), mk(0, 4, 8, True),
                        mk(1, 0, 4, False), mk(1, 4, 8, True)]

            # ---- attention: one global software-pipelined stream --------
            o_cur = {}       # (u, h) -> psum accumulator
            on_t = {}        # u -> o_n2 tile

            def norm_head(u, h):
                """Per-head normalize + ship; emitted right after head h's
                last O matmul of unit u. The multiply reads the PSUM
                accumulator directly (no staging copy)."""
                o_ps = o_cur.pop((u, h))
                last = (u == NU - 1)
                rcp = denpool.tile([1, QC], F32, tag=f"rcp{h}",
                                   name=f"rcp{h}_{u}")
                den = denpool.tile([1, QC], F32, tag=f"den{h}",
                                   name=f"den{h}_{u}")
                nc.vector.tensor_copy(out=den[:], in_=o_ps[64:65, :])
                nc.vector.reciprocal_approx_fast(rcp[:], den[:])
                if last:
                    # no successor reuses this PSUM bank: multiply reads
                    # it directly, skipping the staging copy
                    o_src = o_ps[0:64, :]
                else:
                    nc.vector.tensor_copy(out=stage[:, u * 2 + h, :],
                                          in_=o_ps[0:64, :])
                    o_src = stage[:, u * 2 + h, :]
                rb = denpool.tile([64, QC], F32, tag=f"rb{h}",
                                  name=f"rb{h}_{u}")
                nc.gpsimd.partition_broadcast(rb[:], rcp[:], channels=64)
                o_n2 = on_t[u]
                nc.vector.tensor_tensor(
                    o_n2[h * 64:(h + 1) * 64, :],
                    o_src,
                    rb[:], mybir.AluOpType.mult)
                p, half = u // 2, u % 2
                in_v = a2a_in[p].rearrange(
                    "d p (hh q) -> p d hh q", hh=2)[h * 64:(h + 1) * 64,
                                                    :, half, :]
                nc.sync.dma_start(
                    in_v,
                    o_n2[h * 64:(h + 1) * 64, :].rearrange(
                        "p (d q) -> p d q", d=NCORES))
                if half == 1 and h == 1:
                    nc.gpsimd.collective_compute(
                        "AllToAll", mybir.AluOpType.bypass,
                        replica_groups=GROUPS,
                        ins=[a2a_in[p][:]], outs=[a2a_out[p][:]])
                    c_load(p)

            def emit_O(u, p_t, g):
                bat = u // 4
                for ui, (h, mt) in enumerate(g):
                    nc.tensor.matmul(
                        o_cur[(u, h)][:],
                        v_sb[:, bat * NMT + mt, h * 65:(h + 1) * 65],
                        p_t[:, ui, :],
                        start=(mt == 0), stop=(mt == NMT - 1))
                    if mt == NMT - 1:
                        norm_head(u, h)

            def emit_attention(spsum, opsum, drains, units_range):
                units = [(h, mt) for mt in range(NMT) for h in range(2)]
                pend = []  # 2-deep: S runs 2 groups ahead of O

                def flush():
                    if pend:
                        emit_O(*pend.pop(0))

                for u in units_range:
                    bat = u // 4
                    qcol = u * QC
                    on_t[u] = onpool.tile([128, QC], BF16, tag="on",
                                          name=f"on_{u}")
                    for gi in range(16):
                        g = units[GRP * gi:GRP * (gi + 1)]
                        s_t = spsum.tile([128, GRP, QC], F32, tag="s",
                                         name=f"s_{u}_{gi}")
                        for ui, (h, mt) in enumerate(g):
                            if mt == 0:
                                o_cur[(u, h)] = opsum.tile(
                                    [65, QC], F32, tag=f"o{h}",
                                    name=f"o_{h}_{u}")
                            nc.tensor.matmul(
                                s_t[:, ui, :],
                                kT_sb[h * 64:(h + 1) * 64,
                                      bat * N + mt * 128:
                                      bat * N + (mt + 1) * 128],
                                qT_sb[h * 64:(h + 1) * 64,
                                      qcol:qcol + QC],
                                start=True, stop=True)
                        p_t = ppool.tile([128, GRP, QC], BF16, tag="p",
                                         name=f"p_{u}_{gi}")
                        nc.scalar.activation(p_t[:, 0:GRP, :],
                                             s_t[:, 0:GRP, :], EXP,
                                             scale=SCALE)
                        pend.append((u, p_t, g))
                        if len(pend) > 2:
                            flush()
                        for f in drains.pop((u, gi), []):
                            f()
                while pend:
                    flush()

            # ---- emission schedule --------------------------------------
            drains = {}
            with (
                tc.tile_pool(name="trps", bufs=1, space="PSUM") as trpool,
            ):
                # phase A0-pre: only K(tok 0-511) + Q(u0) up front —
                # attention unit 0 starts after 2 QKV groups. The rest
                # of b0's QKV drains into unit 0's slots, each ahead of
                # the S/O matmuls that consume it.
                with tc.tile_pool(name="aps0", bufs=2,
                                  space="PSUM") as qpool0:
                    for tok0, ft in [(0, 1), (0, 0)]:
                        for f in qkv_subs(qpool0, trpool, 0, tok0, ft):
                            f()

                with (
                    tc.tile_pool(name="sps", bufs=2, space="PSUM") as spsum,
                    tc.tile_pool(name="ops", bufs=1, space="PSUM") as opsum,
                ):
                    with tc.tile_pool(name="aps1", bufs=1,
                                      space="PSUM") as qpool1:
                        # A0-rest drains: K before its S deadline (S
                        # reads key tile mt at group gi=mt), V before
                        # its O deadline, Q before the unit that needs it
                        a0 = {}
                        for tok0, ft in [(0, 2),
                                         (512, 1), (1024, 1), (1536, 1),
                                         (512, 2), (1024, 2), (1536, 2),
                                         (512, 0), (1024, 0), (1536, 0)]:
                            a0[(tok0, ft)] = qkv_subs(qpool1, trpool, 0,
                                                      tok0, ft)
                        # deadlines (2-deep S/O pipeline): a V group for
                        # key tiles 4c..4c+3 must have both subs drained
                        # at slot <= 4c+1 (O for tile mt is emitted at
                        # slot mt+2's flush); K feeding S(gi) at <= gi-1.
                        sched0 = [
                            ((0, 1), (0, 2), 0), ((0, 1), (0, 2), 1),
                            ((0, 2), (512, 1), 0), ((0, 3), (512, 1), 1),
                            ((0, 4), (512, 2), 0), ((0, 5), (512, 2), 1),
                            ((0, 6), (1024, 1), 0), ((0, 7), (1024, 1), 1),
                            ((0, 8), (1024, 2), 0), ((0, 9), (1024, 2), 1),
                            ((0, 10), (1536, 1), 0),
                            ((0, 11), (1536, 1), 1),
                            ((0, 12), (1536, 2), 0),
                            ((0, 13), (1536, 2), 1),
                            ((0, 14), (512, 0), 0), ((0, 15), (512, 0), 1),
                            ((1, 1), (1024, 0), 0), ((1, 2), (1024, 0), 1),
                            ((1, 3), (1536, 0), 0), ((1, 5), (1536, 0), 1),
                        ]
                        for slot, key, si in sched0:
                            drains.setdefault(slot, []).append(a0[key][si])
                        drains.setdefault((0, 3), []).append(
                            lambda: nc.sync.dma_start(
                                x_tiles[2][:], xT_v[:, :, 2 * XCH:3 * XCH]))
                        drains.setdefault((0, 9), []).append(
                            lambda: nc.sync.dma_start(
                                x_tiles[3][:], xT_v[:, :, 3 * XCH:4 * XCH]))
                        # A1 (all of b1's QKV) drains into units 1-4
                        subs = []
                        for tok0, ft in [(0, 1), (512, 1), (1024, 1),
                                         (1536, 1), (0, 0),
                                         (0, 2), (512, 2), (1024, 2),
                                         (1536, 2),
                                         (512, 0), (1024, 0), (1536, 0)]:
                            subs.extend(qkv_subs(qpool1, trpool, 1,
                                                 tok0, ft))
                        # Q(b1,1024/1536) aren't needed until units 6/7:
                        # drain them in unit 5 (EXP-bound, PE headroom)
                        # instead of the PE-bound unit 4
                        slots = ([(1, gi) for gi in (4, 6, 8, 10, 12, 14)]
                                 + [(2, gi) for gi in (1, 3, 5, 7, 9, 11)]
                                 + [(3, gi) for gi in (1, 3, 5, 7, 9, 11)]
                                 + [(4, 1), (4, 3),
                                    (5, 1), (5, 3), (5, 5), (5, 7)])
                        for slot, f in zip(slots, subs):
                            drains.setdefault(slot, []).append(f)
                        # wp_proj weights aren't needed until the out-proj
                        # (unit 6+); loading them here keeps the startup
                        # window's HBM bandwidth for the x chunks
                        drains.setdefault((1, 13), []).append(
                            lambda: nc.sync.dma_start(wp_sb[:], wpT_v[:]))
                        emit_attention(spsum, opsum, drains, range(0, 6))
                    assert not drains

                    with tc.tile_pool(name="cps", bufs=1,
                                      space="PSUM") as cpsum:
        # out-proj: collective completion time is wildly variable
                        # (CC init 60-145us + up to ~35us core stagger),
                        # and a c_mm drain whose A2A hasn't landed stalls
                        # the whole PE FIFO. Only pair 0 (issued ~5 units
                        # earlier) is safe to drain, in unit 7's late
                        # slots; pairs 1-3 run in the tail where a wait
                        # blocks nothing but themselves.
                        for j, f in enumerate(c_mm_subs(0, cpsum)):
                            drains.setdefault((7, 2 * j + 8), []).append(f)
                        emit_attention(spsum, opsum, drains, range(6, 8))
                        assert not drains
                        for pi in (1, 2, 3):
                            for f in c_mm_subs(pi, cpsum):
                                f()
    nc.compile()
    return nc


def kernel(x, w_qkv, w_proj, b_proj):
    global _NC, LAST_EXEC_NS
    if _NC is None:
        _NC = _build()
    x = np.asarray(x, dtype=np.float32)
    w_qkv = np.asarray(w_qkv, dtype=np.float32)
    w_proj = np.asarray(w_proj, dtype=np.float32)
    b_proj = np.asarray(b_proj, dtype=np.float32)

    import ml_dtypes
    xT = np.ascontiguousarray(x.reshape(NT, C).T).astype(ml_dtypes.bfloat16)
    wpT = np.ascontiguousarray(w_proj.T).astype(ml_dtypes.bfloat16)
    bias = np.ascontiguousarray(b_proj.reshape(1, C))
    idn = np.eye(128, dtype=ml_dtypes.bfloat16)
    in_maps = []
    for c in range(NCORES):
        blk = slice(128 * c, 128 * (c + 1))
        wT = np.ascontiguousarray(
            np.concatenate([w_qkv[0:C][blk], w_qkv[C:2 * C][blk],
                            w_qkv[2 * C:3 * C][blk]], axis=0).T).astype(
                ml_dtypes.bfloat16)
        in_maps.append({"xT": xT, "wT": wT, "wpT": wpT, "bias": bias,
                        "idn": idn})

    if TRACE:
        _install_ntff_hook()
    res = run_bass_kernel_spmd(_NC, in_maps, core_ids=list(range(NCORES)),
                               trace=TRACE)
    LAST_EXEC_NS = res.exec_time_ns
    # core c's rows are (pair, 128): row pi*128 + j ->
    # global token (2*pi + j//64)*512 + c*64 + (j%64)
    arr = np.stack([res.results[i]["out"] for i in range(NCORES)])
    out = arr.reshape(NCORES, NU, 64, C).transpose(1, 0, 2, 3)
    return np.ascontiguousarray(
        out.reshape(B, N, C).astype(np.float32))
